# revision 45
# baseline (speedup 1.0000x reference)
"""Trainium2 Bass kernel: gated MSA row attention (AlphaFold-style).

Shapes: q_data/k_data [1,128,256,256], bias [1,8,256,256], k_mask [1,128,256].
Sharding: data-parallel over the 128 sequences -> 16 per core on 8 cores.

Design notes (~173us vs the 212us baseline; ~2.4GHz runs):
- all inputs and constants shipped as bf16 (halves HBM traffic and removes
  all on-chip input casts); constants used directly as matmul operands from
  the packed image; weight pack DMA split from the bias pack DMA and seq 0's
  input prefetched first so projections start ~10us in
- 1/sqrt(dk) folded into Wq host-side so q and k share one psum tile and one
  psum->sbuf cast per m-block
- bias stays as a per-seq PSUM preload via identity matmul: the PE streams
  it at 1 col/cycle, which beats any DVE/GpSimd elementwise alternative and
  keeps the exp->wavg chain short
- logits matmuls from the two half-tiles interleaved: disjoint PE row bands
  (0/32 vs 64/96) and disjoint PSUM banks give 4-way band concurrency
- software-pipelined emission: att(s) -> front(s+1) -> tail(s), so each
  engine's FIFO interleaves seq s+1's projections/casts ahead of seq s's
  tail chain (wsb -> sel -> recip -> gated -> out-proj)
- PSUM rings phase-separated (attention 2x[128,1024] / wavg+out 1x[128,1024]
  / proj+gate 2x[128,512] = exactly 8 banks) so cross-seq WAR couplings stay
  within a phase
- elementwise spread: exp+tanh on scalar, copies+recip+t1 on vector, the
  final gated half and gate01 affine on gpsimd
"""

import os
import sys
import numpy as np
from contextlib import ExitStack

sys.path.insert(0, "/opt/trn_rl_repo")

import concourse.bass as bass
import concourse.bacc as bacc
import concourse.mybir as mybir
from concourse import tile
from concourse.bass_utils import run_bass_kernel_spmd

try:
    from ml_dtypes import bfloat16 as np_bf16
except ImportError:  # pragma: no cover
    np_bf16 = None

NCORES = 8
S = 128
SS = S // NCORES          # 16 sequences per core
L = 256                   # residues (q and k length)
C = 256                   # channels
H = 8                     # heads
DK = 32                   # head dim
SCALE = 1.0 / np.sqrt(DK)
MASK_NEG = -30.0          # additive logit offset for masked keys

F32 = mybir.dt.float32
BF16 = mybir.dt.bfloat16
U8 = mybir.dt.uint8
AF = mybir.ActivationFunctionType

# bf16 pack column offsets
OFF_WQ = 0
OFF_WK = OFF_WQ + 512
OFF_WV = OFF_WK + 512
OFF_WG = OFF_WV + 512
OFF_WO = OFF_WG + 1024
OFF_SEL = OFF_WO + 1024
OFF_ID = OFF_SEL + 128
OFF_BIAS = OFF_ID + 128
NPACKB = OFF_BIAS + 4096
OFF_WEIGHTS_END = OFF_BIAS  # first DMA covers [0, OFF_BIAS)

# head h -> logits/exp block position; block order [h0,h4 | h1,h5 | h2,h6 | h3,h7]
POS = [2 * (h % 4) + (h // 4) for h in range(8)]
HEAD_AT = [0] * 8
for _h in range(8):
    HEAD_AT[POS[_h]] = _h

# head h -> wavg slot: row half 64*(h//4), col block 256*CB[h].  Chosen so the
# wavg pairs (0,5)(2,7)(4,1)(6,3) differ in BOTH PE col-group and PSUM bank,
# allowing 2-way concurrency without tripping the bank-wide has_written clear.
CB = [2 * (h % 2) + ((h % 4) // 2) for h in range(8)]
ROW_HALF = [h // 4 for h in range(8)]
# inverse: col block cb holds heads (rows 0-63, rows 64-127):
HLO = [0, 2, 1, 3]
HHI = [4, 6, 5, 7]
WAVG_PAIRS = [(0, 5), (2, 7), (4, 1), (6, 3)]

_CACHE = {}


def _build_nc():
    nc = bacc.Bacc()

    # q and k inputs packed in one tensor: [seq, (q|k), C, L] -> one DMA/seq
    xqkT_e = nc.declare_dram_parameter("xqkT", [SS, 2, C, L], BF16, isOutput=False)
    maskT_e = nc.declare_dram_parameter("maskT", [128, 2 * SS], U8, isOutput=False)
    packb_e = nc.declare_dram_parameter("packb", [128, NPACKB], BF16, isOutput=False)
    packf_e = nc.declare_dram_parameter("packf", [128, 4], F32, isOutput=False)
    out_e = nc.declare_dram_parameter("out", [SS * L, 256], F32, isOutput=True)

    with ExitStack() as ctx:
        tc = ctx.enter_context(tile.TileContext(nc))

        # ---------------- pools ----------------
        cpool = ctx.enter_context(tc.tile_pool(name="const", bufs=1))
        xpool = ctx.enter_context(tc.tile_pool(name="x", bufs=4))
        qkpool = ctx.enter_context(tc.tile_pool(name="qk", bufs=3))
        gpool = ctx.enter_context(tc.tile_pool(name="g", bufs=3))
        epool = ctx.enter_context(tc.tile_pool(name="e", bufs=4))
        wpool = ctx.enter_context(tc.tile_pool(name="w", bufs=3))
        opool = ctx.enter_context(tc.tile_pool(name="o", bufs=4))
        # PSUM: psl ring (2x[128,1024]=4 banks) carries the 4 attention groups
        # + the sel/denominator tile; psw ring (1x[128,1024]=2 banks) carries
        # wavg + out-proj; early ring (2x[128,512]=2 banks) carries proj/gate.
        # Phase-separated so seq s+1's front half never waits on seq s's tail.
        ps_l = ctx.enter_context(tc.tile_pool(name="psl", bufs=2, space="PSUM"))
        ps_w = ctx.enter_context(tc.tile_pool(name="psw", bufs=1, space="PSUM"))
        ps_fx = ctx.enter_context(tc.tile_pool(name="psfx", bufs=2, space="PSUM"))

        # prefetch seq 0's inputs before the constant packs; q-half triggered
        # on sync, k-half on scalar so the transfers start in parallel
        xqk0 = xpool.tile([128, 4 * L], BF16, tag="xqk", name="xqk")
        nc.sync.dma_start(
            xqk0[:, 0:2 * L].rearrange("p (c l) -> p c l", c=2),
            xqkT_e[0, 0].rearrange("(c p) l -> p c l", c=2))
        nc.scalar.dma_start(
            xqk0[:, 2 * L:4 * L].rearrange("p (c l) -> p c l", c=2),
            xqkT_e[0, 1].rearrange("(c p) l -> p c l", c=2))

        cpack = cpool.tile([128, NPACKB], BF16, name="cpack")
        # q/k weights land first so projections can start; rest streams behind
        nc.sync.dma_start(cpack[:, 0:OFF_WV], packb_e[:, 0:OFF_WV])
        nc.sync.dma_start(cpack[:, OFF_WV:OFF_WEIGHTS_END], packb_e[:, OFF_WV:OFF_WEIGHTS_END])
        nc.sync.dma_start(cpack[:, OFF_BIAS:NPACKB], packb_e[:, OFF_BIAS:NPACKB])
        packf = cpool.tile([128, 4], F32, name="packf")
        nc.sync.dma_start(packf[:], packf_e[:])
        mpack = cpool.tile([128, 2 * SS], U8, name="mpack")
        nc.sync.dma_start(mpack[:], maskT_e[:])

        def wq_sl(kc, m):
            return cpack[:, OFF_WQ + 256 * kc + 128 * m:OFF_WQ + 256 * kc + 128 * (m + 1)]

        def wk_sl(kc, m):
            return cpack[:, OFF_WK + 256 * kc + 128 * m:OFF_WK + 256 * kc + 128 * (m + 1)]

        def wv_sl(kc):
            return cpack[:, OFF_WV + 256 * kc:OFF_WV + 256 * (kc + 1)]

        def wg_sl(kc, t):
            return cpack[:, OFF_WG + 512 * kc + 128 * t:OFF_WG + 512 * kc + 128 * (t + 1)]

        def wo_sl(t):
            return cpack[:, OFF_WO + 256 * t:OFF_WO + 256 * (t + 1)]

        sel_sl = cpack[:, OFF_SEL:OFF_SEL + 128]
        id_sl = cpack[:, OFF_ID:OFF_ID + 128]

        def bias_sl(kc, half, q2):
            off = OFF_BIAS + 2048 * kc + 1024 * half + 512 * q2
            return cpack[:, off:off + 512]

        # mask -> additive offsets [128, SS] per k-chunk: mask*30 - 30
        maskadd_sb = []
        for kc in range(2):
            mf = cpool.tile([128, SS], F32, name=f"maskadd{kc}")
            nc.vector.tensor_scalar(
                mf[:], mpack[:, SS * kc:SS * (kc + 1)], -MASK_NEG, MASK_NEG,
                op0=mybir.AluOpType.mult, op1=mybir.AluOpType.add,
            )
            maskadd_sb.append(mf)

        # persistent v tiles [128, 2*8*64]: kc-chunk x head slot
        # (per head: 32 v-cols | ones x2 | 30 zeros)
        NVB = 2
        v_sb = [cpool.tile([128, 1024], BF16, name=f"vsb{b}") for b in range(NVB)]
        for b in range(NVB):
            t4 = v_sb[b][:].rearrange("p (kc h w) -> p kc h w", kc=2, w=64)
            nc.gpsimd.memset(t4[:, :, :, 34:64], 0.0)
            nc.gpsimd.memset(t4[:, :, :, 32:34], 1.0)

        # HAM pre-warm: ~3.5us of dummy matmuls on scratch data while the
        # first input/weight DMAs are in flight, so the real matmuls start at
        # the 2.4GHz K=8/8 clock instead of paying the cold 1.2GHz window
        warm = cpool.tile([128, 512], BF16, name="warm")
        nc.gpsimd.memset(warm[:], 0.0)
        pwarm = ps_fx.tile([128, 512], F32, tag="fx", name="pwarm")
        for _ in range(8):
            nc.tensor.matmul(pwarm[:], warm[:, 0:128], warm[:],
                             start=True, stop=True, skip_group_check=True)

        def front(s):
            """Input DMA + q/k/v/gate projections for sequence s."""
            if s == 0:
                xqk = xqk0
            else:
                xqk = xpool.tile([128, 4 * L], BF16, tag="xqk", name="xqk")
                nc.sync.dma_start(
                    xqk[:].rearrange("p (qk c l) -> p qk c l", qk=2, c=2),
                    xqkT_e[s].rearrange("qk (c p) l -> p qk c l", c=2))
            xq = [xqk[:, 0:L], xqk[:, L:2 * L]]
            xk = [xqk[:, 2 * L:3 * L], xqk[:, 3 * L:4 * L]]

            qkt = []
            for m in range(2):
                pqk = ps_fx.tile([128, 512], F32, tag="fx", name=f"pqk{m}")
                for kc in range(2):
                    nc.tensor.matmul(
                        pqk[:, 0:256], wq_sl(kc, m), xq[kc],
                        start=(kc == 0), stop=(kc == 1),
                    )
                for kc in range(2):
                    nc.tensor.matmul(
                        pqk[:, 256:512], wk_sl(kc, m), xk[kc],
                        start=(kc == 0), stop=(kc == 1),
                    )
                qk = qkpool.tile([128, 512], BF16, tag=f"qk{m}", name=f"qk{m}")
                nc.vector.tensor_copy(qk[:], pqk[:])
                qkt.append(qk)

            vcur = v_sb[s % NVB]
            pv = ps_fx.tile([128, 512], F32, tag="fx", name="pv")
            for lc in range(2):
                for kc in range(2):
                    nc.tensor.matmul(
                        pv[:, 256 * lc:256 * (lc + 1)],
                        xk[kc][:, 128 * lc:128 * (lc + 1)],
                        wv_sl(kc), start=(kc == 0), stop=(kc == 1),
                    )
            # on scalar: keeps this cast out of the vector FIFO, where it
            # would delay the previous sequence's tail-critical ops
            nc.scalar.copy(
                vcur[:].rearrange("p (lc h w) -> p lc h w", lc=2, w=64)[:, :, :, 0:32],
                pv[:].rearrange("p (lc h w) -> p lc h w", lc=2, w=32),
            )

            gate = gpool.tile([128, 1024], BF16, tag="gate", name="gate")
            for tp in range(2):
                pg2 = ps_fx.tile([128, 512], F32, tag="fx", name=f"pg{tp}")
                for tt in range(2):
                    t = 2 * tp + tt
                    for kc in range(2):
                        nc.tensor.matmul(
                            pg2[:, 256 * tt:256 * (tt + 1)], wg_sl(kc, t), xq[kc],
                            start=(kc == 0), stop=(kc == 1),
                        )
                for tt in range(2):
                    t = 2 * tp + tt
                    nc.scalar.activation(
                        gate[:, 256 * t:256 * (t + 1)], pg2[:, 256 * tt:256 * (tt + 1)],
                        AF.Tanh, bias=packf[:, t:t + 1], scale=0.5,
                    )
            gate01 = gpool.tile([128, 1024], BF16, tag="gate01", name="gate01")
            nc.gpsimd.tensor_scalar(
                gate01[:], gate[:], 0.5, 0.5,
                op0=mybir.AluOpType.mult, op1=mybir.AluOpType.add,
            )
            return dict(qkt=qkt, vcur=vcur, gate01=gate01)

        def att(s, st):
            """Bias preload + banded logits + exp for sequence s.

            The two half-tiles use disjoint PE row bands (0/32 vs 64/96) and
            disjoint PSUM banks, so interleaving their logits matmuls gives
            4-way band concurrency.
            """
            qkt = st["qkt"]
            expT = []
            for kc in range(2):
                e2 = epool.tile([128, H * L], BF16, tag="e2", name=f"exp{kc}")
                pls = []
                for half in range(2):
                    pl = ps_l.tile([128, 1024], F32, tag="pl", name="pl")
                    for q2 in range(2):
                        nc.tensor.matmul(
                            pl[:, 512 * q2:512 * (q2 + 1)], id_sl,
                            bias_sl(kc, half, q2),
                            start=True, stop=False, skip_group_check=True,
                        )
                    pls.append(pl)
                for hh in (0, 2, 1, 3):
                    for half in range(2):
                        h = HEAD_AT[4 * half + hh]
                        m, r = h // 4, 32 * (h % 4)
                        nc.tensor.matmul(
                            pls[half][:, 256 * hh:256 * (hh + 1)],
                            qkt[m][r:r + 32, 256 + 128 * kc:256 + 128 * (kc + 1)],
                            qkt[m][r:r + 32, 0:256], start=False, stop=True,
                            tile_position=(r, 0), skip_group_check=True,
                        )
                for half in range(2):
                    nc.scalar.activation(
                        e2[:, 1024 * half:1024 * (half + 1)], pls[half][:],
                        AF.Exp, bias=maskadd_sb[kc][:, s:s + 1])
                expT.append(e2)
            st["expT"] = expT

        def tail(s, st):
            """wavg + normalize + gate + output projection for sequence s."""
            vcur, gate01, expT = st["vcur"], st["gate01"], st["expT"]
            pw = ps_w.tile([128, 1024], F32, tag="pw", name="pw")

            def wavg_mm(h, kc):
                a = ROW_HALF[h]
                nc.tensor.matmul(
                    pw[64 * a:64 * (a + 1), 256 * CB[h]:256 * (CB[h] + 1)],
                    vcur[:, 512 * kc + 64 * h:512 * kc + 64 * (h + 1)],
                    expT[kc][:, 256 * POS[h]:256 * (POS[h] + 1)],
                    start=(kc == 0), stop=(kc == 1),
                    tile_position=(0, 64 * a), skip_group_check=True,
                )

            for hA, hB in WAVG_PAIRS:
                wavg_mm(hA, 0)
                wavg_mm(hB, 0)
                wavg_mm(hA, 1)
                wavg_mm(hB, 1)
            # psum->sbuf copy on vector (scalar's FIFO carries next-seq tanh,
            # which would delay this tail-critical copy)
            wsb = wpool.tile([128, 1024], BF16, tag="wsb", name="wsb")
            nc.vector.tensor_copy(wsb[:], pw[:])

            # t1 emitted BEFORE the sel/recip pair: it only needs wsb, so the
            # vector engine computes it during the sel matmul + psum latency
            # instead of idling in front of the reciprocal
            t1 = wpool.tile([128, 1024], BF16, tag="t1", name="t1")
            nc.vector.tensor_mul(t1[:], wsb[:], gate01[:])

            # broadcast denominators to all rows, then reciprocal
            recipb = wpool.tile([128, 1024], F32, tag="recipb", name="recipb")
            pdh = ps_w.tile([128, 1024], F32, tag="pw", name="pdh")
            for half in range(2):
                nc.tensor.matmul(pdh[:, 512 * half:512 * (half + 1)], sel_sl,
                                 wsb[:, 512 * half:512 * (half + 1)],
                                 start=True, stop=True)
            nc.vector.reciprocal_approx_fast(recipb[:], pdh[:])

            # gated = wsb * gate01 * recipb; final multiply split v/gpsimd
            gated = wpool.tile([128, 1024], BF16, tag="gated", name="gated")
            nc.vector.tensor_mul(gated[:, 0:512], t1[:, 0:512], recipb[:, 0:512])
            nc.gpsimd.tensor_mul(gated[:, 512:1024], t1[:, 512:1024],
                                 recipb[:, 512:1024])

            # output projection: t outer / lc inner to share the wo weights
            po = ps_w.tile([128, 1024], F32, tag="pw", name="po")
            for t in range(4):
                for lc in range(2):
                    nc.tensor.matmul(
                        po[:, 512 * lc:512 * lc + 256],
                        gated[:, 256 * t + 128 * lc:256 * t + 128 * (lc + 1)],
                        wo_sl(t), start=(t == 0), stop=(t == 3),
                        skip_group_check=True,
                    )
            osb = opool.tile([128, 512], F32, tag="osb", name="osb")
            nc.vector.tensor_copy(
                osb[:].rearrange("p (lc c) -> p lc c", lc=2),
                po[:].rearrange("p (lc z) -> p lc z", lc=2)[:, :, 0:256],
            )
            nc.sync.dma_start(
                out_e[L * s:L * (s + 1), :].rearrange("(lc p) c -> p lc c", lc=2),
                osb[:].rearrange("p (lc c) -> p lc c", lc=2),
            )

        # software-pipelined emission: seq s+1's projections are enqueued
        # before seq s's tail so the per-engine FIFOs interleave them.
        states = {0: front(0)}
        for s in range(SS):
            att(s, states[s])
            if s + 1 < SS:
                states[s + 1] = front(s + 1)
            tail(s, states[s])
            del states[s]

    nc.finalize()
    return nc


def _host_prep(q_data, k_data, bias, k_mask, Wq, Wk, Wv, Wg, bg, Wo, bo):
    """Layout transforms / dtype casts / constant folds on the host."""
    assert np_bf16 is not None, "ml_dtypes required for bf16 host prep"
    q_data = np.asarray(q_data, dtype=np.float32)
    k_data = np.asarray(k_data, dtype=np.float32)
    bias = np.asarray(bias, dtype=np.float32)
    k_mask = np.asarray(k_mask)

    xqkT = np.ascontiguousarray(
        np.stack([q_data[0].transpose(0, 2, 1),
                  k_data[0].transpose(0, 2, 1)], axis=1).astype(np_bf16))  # [S,2,C,L]
    biasT_h = bias[0].transpose(2, 0, 1)          # [k, h, q]
    biasT = np.zeros((L, H * L), np.float32)
    for h in range(H):
        biasT[:, 256 * POS[h]:256 * (POS[h] + 1)] = biasT_h[:, h, :]
    maskT_all = np.ascontiguousarray(k_mask[0].astype(np.uint8).T)  # [L, S]

    Wg_ = np.asarray(Wg, dtype=np.float32)
    Wo_ = np.asarray(Wo, dtype=np.float32)
    bg_ = np.asarray(bg, dtype=np.float32)
    wg_p = np.zeros((C, 512), np.float32)
    wo_p = np.zeros((4, 128, 256), np.float32)
    bg_p = np.zeros((4, 128, 1), np.float32)
    for cb in range(4):
        for a, h in ((0, HLO[cb]), (1, HHI[cb])):
            wg_p[:, 128 * cb + 64 * a:128 * cb + 64 * a + 32] = Wg_[:, 32 * h:32 * h + 32]
            wo_p[cb, 64 * a:64 * a + 32, :] = Wo_[32 * h:32 * h + 32, :]
            bg_p[cb, 64 * a:64 * a + 32, 0] = bg_[32 * h:32 * h + 32]
        bg_p[cb, 33, 0] = 60.0
        bg_p[cb, 97, 0] = 60.0

    wo_p[0, 33, :] = np.asarray(bo, np.float32)
    sel = np.zeros((128, 128), np.float32)
    sel[32, 0:64] = 1.0
    sel[96, 64:128] = 1.0

    packb = np.zeros((128, NPACKB), np.float32)
    Wq_ = np.asarray(Wq, np.float32) * SCALE
    Wk_ = np.asarray(Wk, np.float32)
    Wv_ = np.asarray(Wv, np.float32)
    for kc in range(2):
        packb[:, OFF_WQ + 256 * kc:OFF_WQ + 256 * (kc + 1)] = Wq_[128 * kc:128 * (kc + 1)]
        packb[:, OFF_WK + 256 * kc:OFF_WK + 256 * (kc + 1)] = Wk_[128 * kc:128 * (kc + 1)]
        packb[:, OFF_WV + 256 * kc:OFF_WV + 256 * (kc + 1)] = Wv_[128 * kc:128 * (kc + 1)]
        packb[:, OFF_WG + 512 * kc:OFF_WG + 512 * (kc + 1)] = wg_p[128 * kc:128 * (kc + 1)]
        packb[:, OFF_BIAS + 2048 * kc:OFF_BIAS + 2048 * (kc + 1)] = biasT[128 * kc:128 * (kc + 1)]
    for t in range(4):
        packb[:, OFF_WO + 256 * t:OFF_WO + 256 * (t + 1)] = wo_p[t]
    packb[:, OFF_SEL:OFF_SEL + 128] = sel
    packb[:, OFF_ID:OFF_ID + 128] = np.eye(128, dtype=np.float32)
    packb = packb.astype(np_bf16)

    packf = np.zeros((128, 4), np.float32)
    for t in range(4):
        packf[:, t] = bg_p[t, :, 0] * 0.5

    common = dict(packb=packb, packf=packf)
    in_maps = []
    for i in range(NCORES):
        m = dict(common)
        m["xqkT"] = np.ascontiguousarray(xqkT[SS * i:SS * (i + 1)])
        md = np.zeros((128, 2 * SS), np.uint8)
        mt = maskT_all[:, SS * i:SS * (i + 1)]
        md[:, 0:SS] = mt[0:128]
        md[:, SS:2 * SS] = mt[128:256]
        m["maskT"] = md
        in_maps.append(m)
    return in_maps


def kernel(q_data, k_data, bias, k_mask, Wq, Wk, Wv, Wg, bg, Wo, bo):
    in_maps = _host_prep(q_data, k_data, bias, k_mask, Wq, Wk, Wv, Wg, bg, Wo, bo)
    if "nc" not in _CACHE:
        _CACHE["nc"] = _build_nc()
    trace = bool(int(os.environ.get("KERNEL_TRACE", "0")))
    res = run_bass_kernel_spmd(
        _CACHE["nc"], in_maps, core_ids=list(range(NCORES)), trace=trace,
    )
    _CACHE["last_result"] = res
    out = np.concatenate([res.results[i]["out"] for i in range(NCORES)], axis=0)
    return out.reshape(1, S, L, 256)


# revision 46
# speedup vs baseline: 1.0041x; 1.0041x over previous
"""Trainium2 Bass kernel: gated MSA row attention (AlphaFold-style).

Shapes: q_data/k_data [1,128,256,256], bias [1,8,256,256], k_mask [1,128,256].
Sharding: data-parallel over the 128 sequences -> 16 per core on 8 cores.

Design notes (~173us vs the 212us baseline; ~2.4GHz runs):
- all inputs and constants shipped as bf16 (halves HBM traffic and removes
  all on-chip input casts); constants used directly as matmul operands from
  the packed image; weight pack DMA split from the bias pack DMA and seq 0's
  input prefetched first so projections start ~10us in
- 1/sqrt(dk) folded into Wq host-side so q and k share one psum tile and one
  psum->sbuf cast per m-block
- bias stays as a per-seq PSUM preload via identity matmul: the PE streams
  it at 1 col/cycle, which beats any DVE/GpSimd elementwise alternative and
  keeps the exp->wavg chain short
- logits matmuls from the two half-tiles interleaved: disjoint PE row bands
  (0/32 vs 64/96) and disjoint PSUM banks give 4-way band concurrency
- software-pipelined emission: att(s) -> front(s+1) -> tail(s), so each
  engine's FIFO interleaves seq s+1's projections/casts ahead of seq s's
  tail chain (wsb -> sel -> recip -> gated -> out-proj)
- PSUM rings phase-separated (attention 2x[128,1024] / wavg+out 1x[128,1024]
  / proj+gate 2x[128,512] = exactly 8 banks) so cross-seq WAR couplings stay
  within a phase
- elementwise spread: exp+tanh on scalar, copies+recip+t1 on vector, the
  final gated half and gate01 affine on gpsimd
"""

import os
import sys
import numpy as np
from contextlib import ExitStack

sys.path.insert(0, "/opt/trn_rl_repo")

import concourse.bass as bass
import concourse.bacc as bacc
import concourse.mybir as mybir
from concourse import tile
from concourse.bass_utils import run_bass_kernel_spmd

try:
    from ml_dtypes import bfloat16 as np_bf16
except ImportError:  # pragma: no cover
    np_bf16 = None

NCORES = 8
S = 128
SS = S // NCORES          # 16 sequences per core
L = 256                   # residues (q and k length)
C = 256                   # channels
H = 8                     # heads
DK = 32                   # head dim
SCALE = 1.0 / np.sqrt(DK)
MASK_NEG = -30.0          # additive logit offset for masked keys

F32 = mybir.dt.float32
BF16 = mybir.dt.bfloat16
U8 = mybir.dt.uint8
AF = mybir.ActivationFunctionType

# bf16 pack column offsets
OFF_WQ = 0
OFF_WK = OFF_WQ + 512
OFF_WV = OFF_WK + 512
OFF_WG = OFF_WV + 512
OFF_WO = OFF_WG + 1024
OFF_SEL = OFF_WO + 1024
OFF_ID = OFF_SEL + 128
OFF_BIAS = OFF_ID + 128
NPACKB = OFF_BIAS + 4096
OFF_WEIGHTS_END = OFF_BIAS  # first DMA covers [0, OFF_BIAS)

# head h -> logits/exp block position; block order [h0,h4 | h1,h5 | h2,h6 | h3,h7]
POS = [2 * (h % 4) + (h // 4) for h in range(8)]
HEAD_AT = [0] * 8
for _h in range(8):
    HEAD_AT[POS[_h]] = _h

# head h -> wavg slot: row half 64*(h//4), col block 256*CB[h].  Chosen so the
# wavg pairs (0,5)(2,7)(4,1)(6,3) differ in BOTH PE col-group and PSUM bank,
# allowing 2-way concurrency without tripping the bank-wide has_written clear.
CB = [2 * (h % 2) + ((h % 4) // 2) for h in range(8)]
ROW_HALF = [h // 4 for h in range(8)]
# inverse: col block cb holds heads (rows 0-63, rows 64-127):
HLO = [0, 2, 1, 3]
HHI = [4, 6, 5, 7]
WAVG_PAIRS = [(0, 5), (2, 7), (4, 1), (6, 3)]

_CACHE = {}


def _build_nc():
    nc = bacc.Bacc()

    # q and k inputs packed in one tensor: [seq, (q|k), C, L] -> one DMA/seq
    xqkT_e = nc.declare_dram_parameter("xqkT", [SS, 2, C, L], BF16, isOutput=False)
    maskT_e = nc.declare_dram_parameter("maskT", [128, 2 * SS], U8, isOutput=False)
    packb_e = nc.declare_dram_parameter("packb", [128, NPACKB], BF16, isOutput=False)
    packf_e = nc.declare_dram_parameter("packf", [128, 4], F32, isOutput=False)
    out_e = nc.declare_dram_parameter("out", [SS * L, 256], F32, isOutput=True)

    with ExitStack() as ctx:
        tc = ctx.enter_context(tile.TileContext(nc))

        # ---------------- pools ----------------
        cpool = ctx.enter_context(tc.tile_pool(name="const", bufs=1))
        xpool = ctx.enter_context(tc.tile_pool(name="x", bufs=4))
        qkpool = ctx.enter_context(tc.tile_pool(name="qk", bufs=3))
        gpool = ctx.enter_context(tc.tile_pool(name="g", bufs=3))
        epool = ctx.enter_context(tc.tile_pool(name="e", bufs=4))
        wpool = ctx.enter_context(tc.tile_pool(name="w", bufs=3))
        opool = ctx.enter_context(tc.tile_pool(name="o", bufs=4))
        # PSUM: psl ring (2x[128,1024]=4 banks) carries the 4 attention groups
        # + the sel/denominator tile; psw ring (1x[128,1024]=2 banks) carries
        # wavg + out-proj; early ring (2x[128,512]=2 banks) carries proj/gate.
        # Phase-separated so seq s+1's front half never waits on seq s's tail.
        ps_l = ctx.enter_context(tc.tile_pool(name="psl", bufs=2, space="PSUM"))
        ps_w = ctx.enter_context(tc.tile_pool(name="psw", bufs=1, space="PSUM"))
        ps_fx = ctx.enter_context(tc.tile_pool(name="psfx", bufs=2, space="PSUM"))

        # prefetch seq 0's inputs before the constant packs; q-half triggered
        # on sync, k-half on scalar so the transfers start in parallel
        xqk0 = xpool.tile([128, 4 * L], BF16, tag="xqk", name="xqk")
        nc.sync.dma_start(
            xqk0[:, 0:2 * L].rearrange("p (c l) -> p c l", c=2),
            xqkT_e[0, 0].rearrange("(c p) l -> p c l", c=2))
        nc.scalar.dma_start(
            xqk0[:, 2 * L:4 * L].rearrange("p (c l) -> p c l", c=2),
            xqkT_e[0, 1].rearrange("(c p) l -> p c l", c=2))

        cpack = cpool.tile([128, NPACKB], BF16, name="cpack")
        # q/k weights land first so projections can start; rest streams behind
        nc.sync.dma_start(cpack[:, 0:OFF_WV], packb_e[:, 0:OFF_WV])
        nc.sync.dma_start(cpack[:, OFF_WV:OFF_WEIGHTS_END], packb_e[:, OFF_WV:OFF_WEIGHTS_END])
        nc.sync.dma_start(cpack[:, OFF_BIAS:NPACKB], packb_e[:, OFF_BIAS:NPACKB])
        packf = cpool.tile([128, 4], F32, name="packf")
        nc.sync.dma_start(packf[:], packf_e[:])
        mpack = cpool.tile([128, 2 * SS], U8, name="mpack")
        nc.sync.dma_start(mpack[:], maskT_e[:])

        def wq_sl(kc, m):
            return cpack[:, OFF_WQ + 256 * kc + 128 * m:OFF_WQ + 256 * kc + 128 * (m + 1)]

        def wk_sl(kc, m):
            return cpack[:, OFF_WK + 256 * kc + 128 * m:OFF_WK + 256 * kc + 128 * (m + 1)]

        def wv_sl(kc):
            return cpack[:, OFF_WV + 256 * kc:OFF_WV + 256 * (kc + 1)]

        def wg_sl(kc, t):
            return cpack[:, OFF_WG + 512 * kc + 128 * t:OFF_WG + 512 * kc + 128 * (t + 1)]

        def wo_sl(t):
            return cpack[:, OFF_WO + 256 * t:OFF_WO + 256 * (t + 1)]

        sel_sl = cpack[:, OFF_SEL:OFF_SEL + 128]
        id_sl = cpack[:, OFF_ID:OFF_ID + 128]

        def bias_sl(kc, half, q2):
            off = OFF_BIAS + 2048 * kc + 1024 * half + 512 * q2
            return cpack[:, off:off + 512]

        # mask -> additive offsets [128, SS] per k-chunk: mask*30 - 30
        maskadd_sb = []
        for kc in range(2):
            mf = cpool.tile([128, SS], F32, name=f"maskadd{kc}")
            nc.vector.tensor_scalar(
                mf[:], mpack[:, SS * kc:SS * (kc + 1)], -MASK_NEG, MASK_NEG,
                op0=mybir.AluOpType.mult, op1=mybir.AluOpType.add,
            )
            maskadd_sb.append(mf)

        # HAM pre-warm: ~3us of dummy matmuls on scratch data while the first
        # input/weight DMAs are in flight, so the real matmuls start at the
        # 2.4GHz K=8/8 clock instead of paying the cold 1.2GHz window.
        # The scratch memset is emitted first so it heads gpsimd's queue.
        warm = cpool.tile([128, 512], BF16, name="warm")
        nc.gpsimd.memset(warm[:], 0.0)
        pwarm = ps_fx.tile([128, 512], F32, tag="fx", name="pwarm")
        for _ in range(8):
            nc.tensor.matmul(pwarm[:], warm[:, 0:128], warm[:],
                             start=True, stop=True, skip_group_check=True)

        # persistent v tiles [128, 2*8*64]: kc-chunk x head slot
        # (per head: 32 v-cols | ones x2 | 30 zeros)
        NVB = 2
        v_sb = [cpool.tile([128, 1024], BF16, name=f"vsb{b}") for b in range(NVB)]
        for b in range(NVB):
            t4 = v_sb[b][:].rearrange("p (kc h w) -> p kc h w", kc=2, w=64)
            nc.gpsimd.memset(t4[:, :, :, 34:64], 0.0)
            nc.gpsimd.memset(t4[:, :, :, 32:34], 1.0)

        def front(s):
            """Input DMA + q/k/v/gate projections for sequence s."""
            if s == 0:
                xqk = xqk0
            else:
                xqk = xpool.tile([128, 4 * L], BF16, tag="xqk", name="xqk")
                nc.sync.dma_start(
                    xqk[:].rearrange("p (qk c l) -> p qk c l", qk=2, c=2),
                    xqkT_e[s].rearrange("qk (c p) l -> p qk c l", c=2))
            xq = [xqk[:, 0:L], xqk[:, L:2 * L]]
            xk = [xqk[:, 2 * L:3 * L], xqk[:, 3 * L:4 * L]]

            qkt = []
            for m in range(2):
                pqk = ps_fx.tile([128, 512], F32, tag="fx", name=f"pqk{m}")
                for kc in range(2):
                    nc.tensor.matmul(
                        pqk[:, 0:256], wq_sl(kc, m), xq[kc],
                        start=(kc == 0), stop=(kc == 1),
                    )
                for kc in range(2):
                    nc.tensor.matmul(
                        pqk[:, 256:512], wk_sl(kc, m), xk[kc],
                        start=(kc == 0), stop=(kc == 1),
                    )
                qk = qkpool.tile([128, 512], BF16, tag=f"qk{m}", name=f"qk{m}")
                nc.vector.tensor_copy(qk[:], pqk[:])
                qkt.append(qk)

            vcur = v_sb[s % NVB]
            pv = ps_fx.tile([128, 512], F32, tag="fx", name="pv")
            for lc in range(2):
                for kc in range(2):
                    nc.tensor.matmul(
                        pv[:, 256 * lc:256 * (lc + 1)],
                        xk[kc][:, 128 * lc:128 * (lc + 1)],
                        wv_sl(kc), start=(kc == 0), stop=(kc == 1),
                    )
            # on scalar: keeps this cast out of the vector FIFO, where it
            # would delay the previous sequence's tail-critical ops
            nc.scalar.copy(
                vcur[:].rearrange("p (lc h w) -> p lc h w", lc=2, w=64)[:, :, :, 0:32],
                pv[:].rearrange("p (lc h w) -> p lc h w", lc=2, w=32),
            )

            gate = gpool.tile([128, 1024], BF16, tag="gate", name="gate")
            for tp in range(2):
                pg2 = ps_fx.tile([128, 512], F32, tag="fx", name=f"pg{tp}")
                for tt in range(2):
                    t = 2 * tp + tt
                    for kc in range(2):
                        nc.tensor.matmul(
                            pg2[:, 256 * tt:256 * (tt + 1)], wg_sl(kc, t), xq[kc],
                            start=(kc == 0), stop=(kc == 1),
                        )
                for tt in range(2):
                    t = 2 * tp + tt
                    nc.scalar.activation(
                        gate[:, 256 * t:256 * (t + 1)], pg2[:, 256 * tt:256 * (tt + 1)],
                        AF.Tanh, bias=packf[:, t:t + 1], scale=0.5,
                    )
            gate01 = gpool.tile([128, 1024], BF16, tag="gate01", name="gate01")
            nc.gpsimd.tensor_scalar(
                gate01[:], gate[:], 0.5, 0.5,
                op0=mybir.AluOpType.mult, op1=mybir.AluOpType.add,
            )
            return dict(qkt=qkt, vcur=vcur, gate01=gate01)

        def att(s, st):
            """Bias preload + banded logits + exp for sequence s.

            The two half-tiles use disjoint PE row bands (0/32 vs 64/96) and
            disjoint PSUM banks, so interleaving their logits matmuls gives
            4-way band concurrency.
            """
            qkt = st["qkt"]
            expT = []
            for kc in range(2):
                e2 = epool.tile([128, H * L], BF16, tag="e2", name=f"exp{kc}")
                pls = []
                for half in range(2):
                    pl = ps_l.tile([128, 1024], F32, tag="pl", name="pl")
                    for q2 in range(2):
                        nc.tensor.matmul(
                            pl[:, 512 * q2:512 * (q2 + 1)], id_sl,
                            bias_sl(kc, half, q2),
                            start=True, stop=False, skip_group_check=True,
                        )
                    pls.append(pl)
                for hh in (0, 2, 1, 3):
                    for half in range(2):
                        h = HEAD_AT[4 * half + hh]
                        m, r = h // 4, 32 * (h % 4)
                        nc.tensor.matmul(
                            pls[half][:, 256 * hh:256 * (hh + 1)],
                            qkt[m][r:r + 32, 256 + 128 * kc:256 + 128 * (kc + 1)],
                            qkt[m][r:r + 32, 0:256], start=False, stop=True,
                            tile_position=(r, 0), skip_group_check=True,
                        )
                for half in range(2):
                    nc.scalar.activation(
                        e2[:, 1024 * half:1024 * (half + 1)], pls[half][:],
                        AF.Exp, bias=maskadd_sb[kc][:, s:s + 1])
                expT.append(e2)
            st["expT"] = expT

        def tail(s, st):
            """wavg + normalize + gate + output projection for sequence s."""
            vcur, gate01, expT = st["vcur"], st["gate01"], st["expT"]
            pw = ps_w.tile([128, 1024], F32, tag="pw", name="pw")

            def wavg_mm(h, kc):
                a = ROW_HALF[h]
                nc.tensor.matmul(
                    pw[64 * a:64 * (a + 1), 256 * CB[h]:256 * (CB[h] + 1)],
                    vcur[:, 512 * kc + 64 * h:512 * kc + 64 * (h + 1)],
                    expT[kc][:, 256 * POS[h]:256 * (POS[h] + 1)],
                    start=(kc == 0), stop=(kc == 1),
                    tile_position=(0, 64 * a), skip_group_check=True,
                )

            for hA, hB in WAVG_PAIRS:
                wavg_mm(hA, 0)
                wavg_mm(hB, 0)
                wavg_mm(hA, 1)
                wavg_mm(hB, 1)
            # psum->sbuf copy on vector (scalar's FIFO carries next-seq tanh,
            # which would delay this tail-critical copy)
            wsb = wpool.tile([128, 1024], BF16, tag="wsb", name="wsb")
            nc.vector.tensor_copy(wsb[:], pw[:])

            # t1 emitted BEFORE the sel/recip pair: it only needs wsb, so the
            # vector engine computes it during the sel matmul + psum latency
            # instead of idling in front of the reciprocal
            t1 = wpool.tile([128, 1024], BF16, tag="t1", name="t1")
            nc.vector.tensor_mul(t1[:], wsb[:], gate01[:])

            # broadcast denominators to all rows, then reciprocal
            recipb = wpool.tile([128, 1024], F32, tag="recipb", name="recipb")
            pdh = ps_w.tile([128, 1024], F32, tag="pw", name="pdh")
            for half in range(2):
                nc.tensor.matmul(pdh[:, 512 * half:512 * (half + 1)], sel_sl,
                                 wsb[:, 512 * half:512 * (half + 1)],
                                 start=True, stop=True)
            nc.vector.reciprocal_approx_fast(recipb[:], pdh[:])

            # gated = wsb * gate01 * recipb; final multiply split v/gpsimd
            gated = wpool.tile([128, 1024], BF16, tag="gated", name="gated")
            nc.vector.tensor_mul(gated[:, 0:512], t1[:, 0:512], recipb[:, 0:512])
            nc.gpsimd.tensor_mul(gated[:, 512:1024], t1[:, 512:1024],
                                 recipb[:, 512:1024])

            # output projection: t outer / lc inner to share the wo weights
            po = ps_w.tile([128, 1024], F32, tag="pw", name="po")
            for t in range(4):
                for lc in range(2):
                    nc.tensor.matmul(
                        po[:, 512 * lc:512 * lc + 256],
                        gated[:, 256 * t + 128 * lc:256 * t + 128 * (lc + 1)],
                        wo_sl(t), start=(t == 0), stop=(t == 3),
                        skip_group_check=True,
                    )
            osb = opool.tile([128, 512], F32, tag="osb", name="osb")
            nc.vector.tensor_copy(
                osb[:].rearrange("p (lc c) -> p lc c", lc=2),
                po[:].rearrange("p (lc z) -> p lc z", lc=2)[:, :, 0:256],
            )
            nc.sync.dma_start(
                out_e[L * s:L * (s + 1), :].rearrange("(lc p) c -> p lc c", lc=2),
                osb[:].rearrange("p (lc c) -> p lc c", lc=2),
            )

        # software-pipelined emission: seq s+1's projections are enqueued
        # before seq s's tail so the per-engine FIFOs interleave them.
        states = {0: front(0)}
        for s in range(SS):
            att(s, states[s])
            if s + 1 < SS:
                states[s + 1] = front(s + 1)
            tail(s, states[s])
            del states[s]

    nc.finalize()
    return nc


def _host_prep(q_data, k_data, bias, k_mask, Wq, Wk, Wv, Wg, bg, Wo, bo):
    """Layout transforms / dtype casts / constant folds on the host."""
    assert np_bf16 is not None, "ml_dtypes required for bf16 host prep"
    q_data = np.asarray(q_data, dtype=np.float32)
    k_data = np.asarray(k_data, dtype=np.float32)
    bias = np.asarray(bias, dtype=np.float32)
    k_mask = np.asarray(k_mask)

    xqkT = np.ascontiguousarray(
        np.stack([q_data[0].transpose(0, 2, 1),
                  k_data[0].transpose(0, 2, 1)], axis=1).astype(np_bf16))  # [S,2,C,L]
    biasT_h = bias[0].transpose(2, 0, 1)          # [k, h, q]
    biasT = np.zeros((L, H * L), np.float32)
    for h in range(H):
        biasT[:, 256 * POS[h]:256 * (POS[h] + 1)] = biasT_h[:, h, :]
    maskT_all = np.ascontiguousarray(k_mask[0].astype(np.uint8).T)  # [L, S]

    Wg_ = np.asarray(Wg, dtype=np.float32)
    Wo_ = np.asarray(Wo, dtype=np.float32)
    bg_ = np.asarray(bg, dtype=np.float32)
    wg_p = np.zeros((C, 512), np.float32)
    wo_p = np.zeros((4, 128, 256), np.float32)
    bg_p = np.zeros((4, 128, 1), np.float32)
    for cb in range(4):
        for a, h in ((0, HLO[cb]), (1, HHI[cb])):
            wg_p[:, 128 * cb + 64 * a:128 * cb + 64 * a + 32] = Wg_[:, 32 * h:32 * h + 32]
            wo_p[cb, 64 * a:64 * a + 32, :] = Wo_[32 * h:32 * h + 32, :]
            bg_p[cb, 64 * a:64 * a + 32, 0] = bg_[32 * h:32 * h + 32]
        bg_p[cb, 33, 0] = 60.0
        bg_p[cb, 97, 0] = 60.0

    wo_p[0, 33, :] = np.asarray(bo, np.float32)
    sel = np.zeros((128, 128), np.float32)
    sel[32, 0:64] = 1.0
    sel[96, 64:128] = 1.0

    packb = np.zeros((128, NPACKB), np.float32)
    Wq_ = np.asarray(Wq, np.float32) * SCALE
    Wk_ = np.asarray(Wk, np.float32)
    Wv_ = np.asarray(Wv, np.float32)
    for kc in range(2):
        packb[:, OFF_WQ + 256 * kc:OFF_WQ + 256 * (kc + 1)] = Wq_[128 * kc:128 * (kc + 1)]
        packb[:, OFF_WK + 256 * kc:OFF_WK + 256 * (kc + 1)] = Wk_[128 * kc:128 * (kc + 1)]
        packb[:, OFF_WV + 256 * kc:OFF_WV + 256 * (kc + 1)] = Wv_[128 * kc:128 * (kc + 1)]
        packb[:, OFF_WG + 512 * kc:OFF_WG + 512 * (kc + 1)] = wg_p[128 * kc:128 * (kc + 1)]
        packb[:, OFF_BIAS + 2048 * kc:OFF_BIAS + 2048 * (kc + 1)] = biasT[128 * kc:128 * (kc + 1)]
    for t in range(4):
        packb[:, OFF_WO + 256 * t:OFF_WO + 256 * (t + 1)] = wo_p[t]
    packb[:, OFF_SEL:OFF_SEL + 128] = sel
    packb[:, OFF_ID:OFF_ID + 128] = np.eye(128, dtype=np.float32)
    packb = packb.astype(np_bf16)

    packf = np.zeros((128, 4), np.float32)
    for t in range(4):
        packf[:, t] = bg_p[t, :, 0] * 0.5

    common = dict(packb=packb, packf=packf)
    in_maps = []
    for i in range(NCORES):
        m = dict(common)
        m["xqkT"] = np.ascontiguousarray(xqkT[SS * i:SS * (i + 1)])
        md = np.zeros((128, 2 * SS), np.uint8)
        mt = maskT_all[:, SS * i:SS * (i + 1)]
        md[:, 0:SS] = mt[0:128]
        md[:, SS:2 * SS] = mt[128:256]
        m["maskT"] = md
        in_maps.append(m)
    return in_maps


def kernel(q_data, k_data, bias, k_mask, Wq, Wk, Wv, Wg, bg, Wo, bo):
    in_maps = _host_prep(q_data, k_data, bias, k_mask, Wq, Wk, Wv, Wg, bg, Wo, bo)
    if "nc" not in _CACHE:
        _CACHE["nc"] = _build_nc()
    trace = bool(int(os.environ.get("KERNEL_TRACE", "0")))
    res = run_bass_kernel_spmd(
        _CACHE["nc"], in_maps, core_ids=list(range(NCORES)), trace=trace,
    )
    _CACHE["last_result"] = res
    out = np.concatenate([res.results[i]["out"] for i in range(NCORES)], axis=0)
    return out.reshape(1, S, L, 256)
